# revision 1
# baseline (speedup 1.0000x reference)
"""CRF forward-algorithm loss kernel for Trainium2 (8 NeuronCores, data-parallel over batch).

Math: the reference computes, per batch column b,
    r[b] = logsumexp_tag( alpha_L[b,:] + transition[END,:] ),  L = len[b]
where alpha follows the log-space recurrence
    alpha_{t+1}[next] = logsumexp_prev( alpha_t[prev] + transition[next,prev] ) + feat_t[next]
and the mask freezes alpha once t >= len[b].

We run the recurrence in exp space:  a_t = exp(alpha_t - CZ*t).  CZ is a fixed
per-step log offset that keeps a_t inside fp32 range (per-step growth of alpha
concentrates tightly around log(T) + 1/2 ~ 4.66; cumulative drift over 512
steps has std ~3.7, far inside fp32's e^+-88).

Per-core layout is "packed": 128 partitions = (batch-group g in {0,1}) x (64
tags), free dim = 64 batch columns within the group.  One block-diagonal
128x128 bf16 matmul per step computes P = E @ a for both groups (N=64 moving
columns), then one DVE multiply forms a_{t+1} = P * exp(feat - CZ).

Masking needs no per-step blending: only t = len[b] is ever read.  Each step
t >= TQ0, a second matmul with a one-hot-column weight slice accumulates
q_t = EE . a_t  (EE = exp(transition[END,:])) into row (g*64 + t%64) of a PSUM
block QP += (EE (x) e_row) @ a; rows not selected get += 0.  Blocks of 64 steps
are copied to SBUF, and at the end
    r[b] = sum_t delta_t[b] * log q_t[b] + CZ*len[b]
where delta_t = m[t-1] - m[t] is a host-precomputed one-hot at t = len[b]
(lens are in [256,512], so steps t < TQ0 = 193 skip extraction).  The final
sum over the 64 step-rows is a two-column ones matmul.
"""

import sys

import numpy as np

sys.path.insert(0, "/opt/trn_rl_repo")

S, B, T = 512, 1024, 64
NCORES = 8
BL = B // NCORES   # 128 batch columns per core
G = 2              # batch groups packed on partitions
BG = BL // G       # 64 batch columns per group
CZ = 4.667         # deterministic per-step log offset
TQ0 = 193          # first step with q extraction; 320 rows cover t in [193, 512]
NQB = 5            # q blocks of 64 steps each
BLK = 16           # feat steps per DMA/exp block

_cache: dict = {}
LAST_EXEC_NS = None


def _build():
    import concourse.bacc as bacc
    import concourse.bass as bass
    import concourse.mybir as mybir
    import concourse.tile as tile

    f32 = mybir.dt.float32
    bf16 = mybir.dt.bfloat16
    AF = mybir.ActivationFunctionType

    nc = bacc.Bacc("TRN2", target_bir_lowering=False, debug=False, enable_asserts=False)

    feats_d = nc.dram_tensor("feats_t", (G * T, S, BG), f32, kind="ExternalInput")
    lt2_d = nc.dram_tensor("lt2", (G * T, G * T), f32, kind="ExternalInput")
    ltq2_d = nc.dram_tensor("ltq2", (G * T, 64, G * 64), f32, kind="ExternalInput")
    delta_d = nc.dram_tensor("delta", (G * 64, NQB, BG), f32, kind="ExternalInput")
    tw_d = nc.dram_tensor("tw", (G, BG), f32, kind="ExternalInput")
    out_d = nc.dram_tensor("out", (G, BG), f32, kind="ExternalOutput")

    P128 = G * T  # 128

    with tile.TileContext(nc) as tc:
        with (
            tc.tile_pool(name="const", bufs=1) as cpool,
            tc.tile_pool(name="feat", bufs=3) as fpool,
            tc.tile_pool(name="ef", bufs=3) as efpool,
            tc.tile_pool(name="a", bufs=3) as apool,
            tc.tile_pool(name="acc", bufs=1) as accpool,
            tc.tile_pool(name="pp", bufs=4, space=bass.MemorySpace.PSUM) as ppool,
            tc.tile_pool(name="qp", bufs=2, space=bass.MemorySpace.PSUM) as qpool,
            tc.tile_pool(name="rp", bufs=1, space=bass.MemorySpace.PSUM) as rpool,
        ):
            bias0 = cpool.tile([P128, 1], f32, tag="bias0")
            nc.vector.memset(bias0[:], 0.0)
            biasz = cpool.tile([P128, 1], f32, tag="biasz")
            nc.vector.memset(biasz[:], -CZ)

            # block-diag transition weights (log-space in DRAM, exp'd to bf16 here)
            lt2_log = cpool.tile([P128, P128], f32, tag="lt2_log")
            nc.sync.dma_start(lt2_log[:], lt2_d[:])
            lt2 = cpool.tile([P128, P128], bf16, tag="lt2")
            nc.scalar.activation(lt2[:], lt2_log[:], AF.Exp, bias=bias0[:])

            # one-hot-column q-extraction weights: ltq2[:, kk, :] has EE in col g*64+kk
            ltq2_log = cpool.tile([P128, 64, G * 64], f32, tag="ltq2_log")
            nc.sync.dma_start(ltq2_log[:], ltq2_d[:])
            ltq2 = cpool.tile([P128, 64, G * 64], bf16, tag="ltq2")
            nc.scalar.activation(ltq2[:], ltq2_log[:], AF.Exp, bias=bias0[:])

            delta = cpool.tile([G * 64, NQB, BG], f32, tag="delta")
            nc.sync.dma_start(delta[:], delta_d[:])
            tw = cpool.tile([G, BG], f32, tag="tw")
            nc.sync.dma_start(tw[:], tw_d[:])
            # two-column group-sum weights: col g = indicator(partition in group g)
            onesg = cpool.tile([P128, G], f32, tag="onesg")
            nc.vector.memset(onesg[:], 0.0)
            nc.vector.memset(onesg[0:64, 0:1], 1.0)
            nc.vector.memset(onesg[64:128, 1:2], 1.0)

            qsave = accpool.tile([G * 64, NQB, BG], f32, tag="qsave")

            a = apool.tile([P128, BG], bf16, tag="a")
            nc.vector.memset(a[:], 0.0)
            nc.vector.memset(a[0:1, :], 1.0)
            nc.vector.memset(a[64:65, :], 1.0)

            qblk = None
            for blk in range(S // BLK):
                t0 = blk * BLK
                fb = fpool.tile([P128, BLK, BG], f32, tag="fb")
                nc.sync.dma_start(fb[:], feats_d[:, t0 : t0 + BLK, :])
                ef = efpool.tile([P128, BLK, BG], bf16, tag="ef")
                nc.scalar.activation(ef[:], fb[:], AF.Exp, bias=biasz[:])
                for k in range(BLK):
                    t = t0 + k
                    if t >= TQ0:
                        jj, kk = divmod(t - TQ0, 64)
                        if kk == 0:
                            qblk = qpool.tile([G * 64, BG], f32, tag="qblk")
                        nc.tensor.matmul(
                            qblk[:], ltq2[:, kk, :], a[:],
                            start=(kk == 0), stop=(kk == 63),
                            skip_group_check=True,
                        )
                        if kk == 63:
                            nc.vector.tensor_copy(qsave[:, jj, :], qblk[:])
                    p = ppool.tile([P128, BG], f32, tag="p")
                    nc.tensor.matmul(p[:], lt2[:], a[:], start=True, stop=True)
                    anew = apool.tile([P128, BG], bf16, tag="a")
                    nc.vector.tensor_mul(anew[:], p[:], ef[:, k, :])
                    a = anew

            # q row for t = 512 (block 4, row 63), then flush block 4
            nc.tensor.matmul(
                qblk[:], ltq2[:, 63, :], a[:],
                start=False, stop=True, skip_group_check=True,
            )
            nc.vector.tensor_copy(qsave[:, NQB - 1, :], qblk[:])

            logq = accpool.tile([G * 64, NQB, BG], f32, tag="logq")
            nc.scalar.activation(logq[:], qsave[:], AF.Ln, bias=bias0[:])
            r1 = accpool.tile([G * 64, NQB, BG], f32, tag="r1")
            nc.vector.tensor_mul(r1[:], logq[:], delta[:])

            rsum = rpool.tile([G, BG], f32, tag="rsum")
            for j in range(NQB):
                nc.tensor.matmul(
                    rsum[:], onesg[:], r1[:, j, :],
                    start=(j == 0), stop=(j == NQB - 1),
                )
            rout = accpool.tile([G, BG], f32, tag="rout")
            nc.vector.tensor_add(rout[:], rsum[:], tw[:])
            nc.sync.dma_start(out_d[:], rout[:])

    nc.compile()
    return nc


def _prep_inputs(feats, mask, transition):
    feats = np.asarray(feats, dtype=np.float32)
    mask = np.asarray(mask, dtype=np.float32)
    transition = np.asarray(transition, dtype=np.float32)

    lens = mask.sum(axis=0)  # (B,)
    m_pad = np.concatenate([mask, np.zeros((1, B), np.float32)], axis=0)
    # delta rows r = g*64 + kk, block j: t = TQ0 + 64*j + kk
    tt = TQ0 + 64 * np.arange(NQB)[None, :] + np.arange(64)[:, None]  # [64, NQB]
    delta_full = m_pad[tt - 1, :] - m_pad[tt, :]  # [64, NQB, B]

    NEG = -10000.0
    # block-diagonal log weights: lt2_log[g*64+p, g'*64+n] = trans[n,p] if g==g' else NEG
    lt2_log = np.full((G * T, G * T), NEG, np.float32)
    for g in range(G):
        lt2_log[g * T : (g + 1) * T, g * T : (g + 1) * T] = transition.T
    # one-hot q weights (log space): ltq2_log[g*64+p, kk, m] = trans[END,p] if m==g*64+kk
    ltq2_log = np.full((G * T, 64, G * 64), NEG, np.float32)
    idx = np.arange(64)
    for g in range(G):
        ltq2_log[g * T : (g + 1) * T, idx, g * 64 + idx] = transition[1, :][:, None]

    in_maps = []
    for c in range(NCORES):
        sl = slice(c * BL, (c + 1) * BL)
        fc = feats[:, sl, :]  # (S, BL, T)
        # packed layout [(g*64+tag), t, b']
        fp = np.ascontiguousarray(
            fc.reshape(S, G, BG, T).transpose(1, 3, 0, 2).reshape(G * T, S, BG)
        )
        dc = delta_full[:, :, sl]  # [64, NQB, BL]
        dpacked = np.ascontiguousarray(
            dc.reshape(64, NQB, G, BG).transpose(2, 0, 1, 3).reshape(G * 64, NQB, BG)
        )
        in_maps.append(
            {
                "feats_t": fp,
                "lt2": lt2_log,
                "ltq2": ltq2_log,
                "delta": dpacked,
                "tw": np.ascontiguousarray(
                    (CZ * lens[sl]).astype(np.float32).reshape(G, BG)
                ),
            }
        )
    return in_maps


def kernel(feats, mask, transition, trace=False):
    global LAST_EXEC_NS
    if "nc" not in _cache:
        _cache["nc"] = _build()
    nc = _cache["nc"]

    in_maps = _prep_inputs(feats, mask, transition)

    from concourse.bass_utils import run_bass_kernel_spmd

    res = run_bass_kernel_spmd(nc, in_maps, core_ids=list(range(NCORES)), trace=trace)
    LAST_EXEC_NS = res.exec_time_ns
    out = np.concatenate([r["out"].reshape(BL) for r in res.results], axis=0)
    return out.astype(np.float32)



# revision 2
# speedup vs baseline: 2.0458x; 2.0458x over previous
"""CRF forward-algorithm loss kernel for Trainium2 (8 NeuronCores, data-parallel over batch).

Math: per batch column b, r[b] = logsumexp_tag(alpha_L + transition[END,:]), L = len[b],
with the log-space recurrence alpha_{t}[next] = lse_prev(alpha_{t-1} + trans[next,prev]) + feat_{t-1}.
In exp space (a_t = exp(alpha_t - CZ*t), D_t = diag(exp(feat_{t-1} - CZ))):
    a_t = D_t E a_{t-1},   r[b] = log(w^T a_{len[b]}) + CZ*len[b],  w = exp(trans[END,:]).

The serial chain is halved by meeting in the middle: since len[b] in [256, 512],
    Q[b] = sum_t delta_t[b] * (w^T a_t[b]) = v_256^T a_256       (delta_t = 1 at t = len[b])
where the FORWARD chain computes a_256 (256 steps) and the BACKWARD adjoint chain
    v_512 = delta_512 w;   v_{t-1} = E^T (ef_t * v_t) + delta_{t-1} w,  t = 512..257
computes v_256 (256 steps).  Both chains run concurrently, interleaved on the
Tensor/DVE engines, so the latency-bound step chain (mm -> sem -> mul -> sem,
~650ns) is traversed 256 times instead of 512.

Per-core layout packs 128 partitions = (batch-group g in {0,1}) x (64 tags) with 64
batch columns in the free dim; both chains use one 128x128 block-diagonal bf16
matmul per step.  The delta injection enters the backward chain as a rank-1 (K=2)
matmul accumulated into the same PSUM bank before the main matmul (start=True on the
injection, stop=True on the main), so it stays off the serial dependency chain.
Finally r = ln(sum_tags v_256 * a_256) + CZ*len via a ones-matmul partition reduce.
"""

import sys

import numpy as np

sys.path.insert(0, "/opt/trn_rl_repo")

S, B, T = 512, 1024, 64
NCORES = 8
BL = B // NCORES   # 128 batch columns per core
G = 2              # batch groups packed on partitions
BG = BL // G       # 64 batch columns per group
CZ = 4.667         # deterministic per-step log offset
M = 256            # meeting point of the two chains
BLK = 16           # feat steps per DMA/exp block
NEG = -10000.0

_cache: dict = {}
LAST_EXEC_NS = None


def _build():
    import concourse.bacc as bacc
    import concourse.bass as bass
    import concourse.mybir as mybir
    import concourse.tile as tile

    f32 = mybir.dt.float32
    bf16 = mybir.dt.bfloat16
    AF = mybir.ActivationFunctionType

    nc = bacc.Bacc("TRN2", target_bir_lowering=False, debug=False, enable_asserts=False)

    feats_d = nc.dram_tensor("feats_t", (G * T, S, BG), bf16, kind="ExternalInput")
    wf_d = nc.dram_tensor("wf_log", (G * T, G * T), f32, kind="ExternalInput")
    wb_d = nc.dram_tensor("wb_log", (G * T, G * T), f32, kind="ExternalInput")
    wy_d = nc.dram_tensor("wy_log", (G, G * T), f32, kind="ExternalInput")
    dr_d = nc.dram_tensor("drows", (G, M, BG), bf16, kind="ExternalInput")
    u0_d = nc.dram_tensor("u0", (G * T, BG), bf16, kind="ExternalInput")
    tw_d = nc.dram_tensor("tw", (G, BG), f32, kind="ExternalInput")
    out_d = nc.dram_tensor("out", (G, BG), f32, kind="ExternalOutput")

    P128 = G * T  # 128

    with tile.TileContext(nc) as tc:
        with (
            tc.tile_pool(name="const", bufs=1) as cpool,
            tc.tile_pool(name="ffeat", bufs=3) as ffpool,
            tc.tile_pool(name="fef", bufs=3) as fepool,
            tc.tile_pool(name="bfeat", bufs=3) as bfpool,
            tc.tile_pool(name="bef", bufs=3) as bepool,
            tc.tile_pool(name="a", bufs=3) as apool,
            tc.tile_pool(name="u", bufs=3) as upool,
            tc.tile_pool(name="acc", bufs=1) as accpool,
            tc.tile_pool(name="fps", bufs=3, space=bass.MemorySpace.PSUM) as fpsum,
            tc.tile_pool(name="bps", bufs=3, space=bass.MemorySpace.PSUM) as bpsum,
            tc.tile_pool(name="rp", bufs=1, space=bass.MemorySpace.PSUM) as rpool,
        ):
            bias0 = cpool.tile([P128, 1], f32, tag="bias0")
            nc.vector.memset(bias0[:], 0.0)
            biasz = cpool.tile([P128, 1], f32, tag="biasz")
            nc.vector.memset(biasz[:], -CZ)
            bias0g = cpool.tile([G, 1], f32, tag="bias0g")
            nc.vector.memset(bias0g[:], 0.0)

            # block-diag transition weights (log-space in DRAM, exp'd to bf16 here)
            wf_log = cpool.tile([P128, P128], f32, tag="wf_log")
            nc.sync.dma_start(wf_log[:], wf_d[:])
            wf = cpool.tile([P128, P128], bf16, tag="wf")
            nc.scalar.activation(wf[:], wf_log[:], AF.Exp, bias=bias0[:])

            wb_log = cpool.tile([P128, P128], f32, tag="wb_log")
            nc.sync.dma_start(wb_log[:], wb_d[:])
            wb = cpool.tile([P128, P128], bf16, tag="wb")
            nc.scalar.activation(wb[:], wb_log[:], AF.Exp, bias=bias0[:])

            wy_log = cpool.tile([G, P128], f32, tag="wy_log")
            nc.sync.dma_start(wy_log[:], wy_d[:])
            wy = cpool.tile([G, P128], bf16, tag="wy")
            nc.scalar.activation(wy[:], wy_log[:], AF.Exp, bias=bias0g[:])

            drs = cpool.tile([G, M, BG], bf16, tag="drs")
            nc.sync.dma_start(drs[:], dr_d[:])
            tw = cpool.tile([G, BG], f32, tag="tw")
            nc.sync.dma_start(tw[:], tw_d[:])

            # two-column group-sum weights: col g = indicator(partition in group g)
            onesg = cpool.tile([P128, G], bf16, tag="onesg")
            nc.vector.memset(onesg[:], 0.0)
            nc.vector.memset(onesg[0:T, 0:1], 1.0)
            nc.vector.memset(onesg[T : 2 * T, 1:2], 1.0)

            a_t = apool.tile([P128, BG], bf16, tag="a")
            nc.vector.memset(a_t[:], 0.0)
            nc.vector.memset(a_t[0:1, :], 1.0)
            nc.vector.memset(a_t[T : T + 1, :], 1.0)

            u_t = upool.tile([P128, BG], bf16, tag="u")
            nc.sync.dma_start(u_t[:], u0_d[:])

            v256 = None
            for blk in range(M // BLK):
                t0f = blk * BLK
                fbf = ffpool.tile([P128, BLK, BG], bf16, tag="fbf")
                nc.sync.dma_start(fbf[:], feats_d[:, t0f : t0f + BLK, :])
                eff = fepool.tile([P128, BLK, BG], bf16, tag="eff")
                nc.scalar.activation(eff[:], fbf[:], AF.Exp, bias=biasz[:])

                t0b = 495 - blk * BLK
                fbb = bfpool.tile([P128, BLK, BG], bf16, tag="fbb")
                nc.sync.dma_start(fbb[:], feats_d[:, t0b : t0b + BLK, :])
                efb = bepool.tile([P128, BLK, BG], bf16, tag="efb")
                nc.scalar.activation(efb[:], fbb[:], AF.Exp, bias=biasz[:])

                for k in range(BLK):
                    s = blk * BLK + k
                    # forward step t = s+1: a <- ef_t * (E @ a)
                    pf = fpsum.tile([P128, BG], f32, tag="pf")
                    nc.tensor.matmul(pf[:], wf[:], a_t[:], start=True, stop=True)
                    a_new = apool.tile([P128, BG], bf16, tag="a")
                    nc.vector.tensor_mul(a_new[:], pf[:], eff[:, k, :])
                    a_t = a_new

                    # backward step t = 512-s: v_{t-1} = E^T u_t + delta_{t-1} w
                    vp = bpsum.tile([P128, BG], f32, tag="vp")
                    nc.tensor.matmul(
                        vp[:], wy[:], drs[:, s, :],
                        start=True, stop=False, skip_group_check=True,
                    )
                    nc.tensor.matmul(
                        vp[:], wb[:], u_t[:],
                        start=False, stop=True, skip_group_check=True,
                    )
                    if s < M - 1:
                        u_new = upool.tile([P128, BG], bf16, tag="u")
                        nc.vector.tensor_mul(u_new[:], vp[:], efb[:, BLK - 1 - k, :])
                        u_t = u_new
                    else:
                        v256 = vp  # v_256 stays in PSUM for the epilogue

            # Q = sum_tags(v256 * a256) per column, r = ln Q + CZ*len
            qprod = accpool.tile([P128, BG], bf16, tag="qprod")
            nc.vector.tensor_mul(qprod[:], v256[:], a_t[:])
            rs = rpool.tile([G, BG], f32, tag="rs")
            nc.tensor.matmul(rs[:], onesg[:], qprod[:], start=True, stop=True)
            logq = accpool.tile([G, BG], f32, tag="logq")
            nc.scalar.activation(logq[:], rs[:], AF.Ln, bias=bias0g[:])
            rout = accpool.tile([G, BG], f32, tag="rout")
            nc.vector.tensor_add(rout[:], logq[:], tw[:])
            nc.sync.dma_start(out_d[:], rout[:])

    nc.compile()
    return nc


def _prep_inputs(feats, mask, transition):
    import ml_dtypes

    bf = ml_dtypes.bfloat16
    feats = np.asarray(feats, dtype=np.float32)
    mask = np.asarray(mask, dtype=np.float32)
    transition = np.asarray(transition, dtype=np.float32)

    lens = mask.sum(axis=0)  # (B,)
    m_pad = np.concatenate([mask, np.zeros((1, B), np.float32)], axis=0)
    # d[t] = m[t-1] - m[t] = 1 iff len == t, for t = 1..512
    d = m_pad[np.arange(S + 1) - 1, :] - m_pad[np.arange(S + 1), :]  # d[0] garbage
    d[0] = 0.0

    # block-diagonal log weights
    wf_log = np.full((G * T, G * T), NEG, np.float32)   # fwd: wf[g*T+prev, g*T+next] = trans[next, prev]
    wb_log = np.full((G * T, G * T), NEG, np.float32)   # bwd: wb[g*T+next, g*T+prev] = trans[next, prev]
    for g in range(G):
        wf_log[g * T : (g + 1) * T, g * T : (g + 1) * T] = transition.T
        wb_log[g * T : (g + 1) * T, g * T : (g + 1) * T] = transition
    # rank-1 injection weights: wy[g, g*T+prev] = trans[END=1, prev]
    wy_log = np.full((G, G * T), NEG, np.float32)
    for g in range(G):
        wy_log[g, g * T : (g + 1) * T] = transition[1, :]

    w = np.exp(transition[1, :].astype(np.float64))          # (T,)
    ef_last = np.exp(feats[S - 1].astype(np.float64) - CZ)   # (B, T)
    u0_full = (ef_last * d[S][:, None] * w[None, :]).astype(np.float32)  # (B, T)

    in_maps = []
    for c in range(NCORES):
        sl = slice(c * BL, (c + 1) * BL)
        fc = feats[:, sl, :]  # (S, BL, T)
        fp = np.ascontiguousarray(
            fc.reshape(S, G, BG, T).transpose(1, 3, 0, 2).reshape(G * T, S, BG)
        ).astype(bf)
        # drows[g, j, c'] = d[511 - j][batch col], j = 0..255
        dc = d[511 - np.arange(M), :][:, sl]  # (M, BL)
        dr = np.ascontiguousarray(
            dc.reshape(M, G, BG).transpose(1, 0, 2)
        ).astype(bf)
        u0c = np.ascontiguousarray(
            u0_full[sl].reshape(G, BG, T).transpose(0, 2, 1).reshape(G * T, BG)
        ).astype(bf)
        in_maps.append(
            {
                "feats_t": fp,
                "wf_log": wf_log,
                "wb_log": wb_log,
                "wy_log": wy_log,
                "drows": dr,
                "u0": u0c,
                "tw": np.ascontiguousarray(
                    (CZ * lens[sl]).astype(np.float32).reshape(G, BG)
                ),
            }
        )
    return in_maps


def kernel(feats, mask, transition, trace=False):
    global LAST_EXEC_NS
    if "nc" not in _cache:
        _cache["nc"] = _build()
    nc = _cache["nc"]

    in_maps = _prep_inputs(feats, mask, transition)

    from concourse.bass_utils import run_bass_kernel_spmd

    res = run_bass_kernel_spmd(nc, in_maps, core_ids=list(range(NCORES)), trace=trace)
    LAST_EXEC_NS = res.exec_time_ns
    out = np.concatenate([r["out"].reshape(BL) for r in res.results], axis=0)
    return out.astype(np.float32)


# revision 7
# speedup vs baseline: 2.2252x; 1.0877x over previous
"""CRF forward-algorithm loss kernel for Trainium2 (8 NeuronCores, data-parallel over batch).

Math: per batch column b, r[b] = logsumexp_tag(alpha_L + transition[END,:]), L = len[b],
with the log-space recurrence alpha_{t}[next] = lse_prev(alpha_{t-1} + trans[next,prev]) + feat_{t-1}.
In exp space (a_t = exp(alpha_t - CZ*t), D_t = diag(exp(feat_{t-1} - CZ))):
    a_t = D_t E a_{t-1},   r[b] = log(w^T a_{len[b]}) + CZ*len[b],  w = exp(trans[END,:]).

The serial chain is halved by meeting in the middle: since len[b] in [256, 512],
    Q[b] = sum_t delta_t[b] * (w^T a_t[b]) = v_256^T a_256       (delta_t = 1 at t = len[b])
where the FORWARD chain computes a_256 (256 steps) and the BACKWARD adjoint chain
    v_512 = delta_512 w;   v_{t-1} = E^T (ef_t * v_t) + delta_{t-1} w,  t = 512..257
computes v_256 (256 steps).  Both chains run concurrently, interleaved on the
Tensor/DVE engines; each round is latency-bound (mm -> sem -> mul -> sem ~527ns)
and traversed 256 times instead of 512.

delta-injection rides INSIDE the backward matmul: tags 0 (START) and 1 (END) are
dead in both chains (transition row START / col END are -1e4), so per group the
u-tile row (g,tag=0) is repurposed as a delta-carrier and row (g,tag=1) as a
ones-carrier.  wb's contraction row k=0 holds the injection weights w[prev], row
k=1 is a self-loop pair keeping both carriers alive, and the delta data itself is
delivered through feats rows 0/1 (feat[t,c,0] = CZ if len[c]==t else -1e4;
feat[t,c,1] = CZ), so exp(feat-CZ) regenerates {delta, 1} exactly.  CZ = 4.6875 is
bf16-exact so the ones-carrier survives exp() exactly.  The epilogue group-sum
weights exclude the carrier rows: r = ln(sum_{tags>=2} v_256*a_256) + CZ*len.
"""

import sys

import numpy as np

sys.path.insert(0, "/opt/trn_rl_repo")

S, B, T = 512, 1024, 64
NCORES = 8
BL = B // NCORES   # 128 batch columns per core
G = 2              # batch groups packed on partitions
BG = BL // G       # 64 batch columns per group
CZ = 4.6875        # per-step log offset, exactly representable in bf16
M = 256            # meeting point of the two chains
BLK = 16           # feat steps per DMA/exp block
NEG = -10000.0
PROBE = False      # pool tensor_mul probe crashes the walrus backend; keep off

_cache: dict = {}
LAST_EXEC_NS = None


def _build():
    import concourse.bacc as bacc
    import concourse.bass as bass
    import concourse.mybir as mybir
    import concourse.tile as tile

    f32 = mybir.dt.float32
    bf16 = mybir.dt.bfloat16
    AF = mybir.ActivationFunctionType

    nc = bacc.Bacc("TRN2", target_bir_lowering=False, debug=False, enable_asserts=False)

    feats_d = nc.dram_tensor("feats_t", (G * T, S, BG), bf16, kind="ExternalInput")
    wf_d = nc.dram_tensor("wf_log", (G * T, G * T), f32, kind="ExternalInput")
    wb_d = nc.dram_tensor("wb_log", (G * T, G * T), f32, kind="ExternalInput")
    u0_d = nc.dram_tensor("u0", (G * T, BG), bf16, kind="ExternalInput")
    tw_d = nc.dram_tensor("tw", (G, BG), f32, kind="ExternalInput")
    out_d = nc.dram_tensor("out", (G, BG), f32, kind="ExternalOutput")

    P128 = G * T  # 128

    with tile.TileContext(nc) as tc:
        with (
            tc.tile_pool(name="const", bufs=1) as cpool,
            tc.tile_pool(name="ffeat", bufs=3) as ffpool,
            tc.tile_pool(name="fef", bufs=3) as fepool,
            tc.tile_pool(name="bfeat", bufs=3) as bfpool,
            tc.tile_pool(name="bef", bufs=3) as bepool,
            tc.tile_pool(name="a", bufs=3) as apool,
            tc.tile_pool(name="u", bufs=3) as upool,
            tc.tile_pool(name="acc", bufs=1) as accpool,
            tc.tile_pool(name="fps", bufs=3, space=bass.MemorySpace.PSUM) as fpsum,
            tc.tile_pool(name="bps", bufs=3, space=bass.MemorySpace.PSUM) as bpsum,
            tc.tile_pool(name="rp", bufs=1, space=bass.MemorySpace.PSUM) as rpool,
        ):
            bias0 = cpool.tile([P128, 1], f32, tag="bias0")
            nc.vector.memset(bias0[:], 0.0)
            biasz = cpool.tile([P128, 1], f32, tag="biasz")
            nc.vector.memset(biasz[:], -CZ)
            bias0g = cpool.tile([G, 1], f32, tag="bias0g")
            nc.vector.memset(bias0g[:], 0.0)

            # block-diag transition weights (log-space in DRAM, exp'd to bf16 here)
            wf_log = cpool.tile([P128, P128], f32, tag="wf_log")
            nc.sync.dma_start(wf_log[:], wf_d[:])
            wf = cpool.tile([P128, P128], bf16, tag="wf")
            nc.scalar.activation(wf[:], wf_log[:], AF.Exp, bias=bias0[:])

            wb_log = cpool.tile([P128, P128], f32, tag="wb_log")
            nc.sync.dma_start(wb_log[:], wb_d[:])
            wb = cpool.tile([P128, P128], bf16, tag="wb")
            nc.scalar.activation(wb[:], wb_log[:], AF.Exp, bias=bias0[:])

            tw = cpool.tile([G, BG], f32, tag="tw")
            nc.sync.dma_start(tw[:], tw_d[:])

            # group-sum weights excluding the carrier rows (tags 0, 1 of each group)
            onesg = cpool.tile([P128, G], bf16, tag="onesg")
            nc.vector.memset(onesg[:], 0.0)
            nc.vector.memset(onesg[0:T, 0:1], 1.0)
            nc.vector.memset(onesg[T : 2 * T, 1:2], 1.0)
            # zero the carrier rows (partition offsets must be 0 mod 32)
            nc.vector.memset(onesg[0:2, 0:1], 0.0)
            nc.vector.memset(onesg[T : T + 2, 1:2], 0.0)

            a_t = apool.tile([P128, BG], bf16, tag="a")
            nc.vector.memset(a_t[:], 0.0)
            nc.vector.memset(a_t[0:1, :], 1.0)
            nc.vector.memset(a_t[T : T + 1, :], 1.0)

            u_t = upool.tile([P128, BG], bf16, tag="u")
            nc.sync.dma_start(u_t[:], u0_d[:])

            if PROBE:
                # timing probes: pool vs vector tensor_mul reading PSUM
                pmm = fpsum.tile([P128, BG], f32, tag="pf")
                nc.tensor.matmul(pmm[:], wf[:], a_t[:], start=True, stop=True)
                pin = cpool.tile([P128, BG], bf16, tag="pin")
                nc.vector.memset(pin[:], 1.0)
                pout = cpool.tile([P128, BG], bf16, tag="pout")
                for _ in range(3):
                    nc.gpsimd.tensor_mul(pout[:], pmm[:], pin[:])
                for _ in range(3):
                    nc.vector.tensor_mul(pout[:], pmm[:], pin[:])

            v256 = None
            for blk in range(M // BLK):
                t0f = blk * BLK
                fbf = ffpool.tile([P128, BLK, BG], bf16, tag="fbf")
                nc.sync.dma_start(fbf[:], feats_d[:, t0f : t0f + BLK, :])
                eff = fepool.tile([P128, BLK, BG], bf16, tag="eff")
                nc.scalar.activation(eff[:], fbf[:], AF.Exp, bias=biasz[:])

                t0b = 495 - blk * BLK
                fbb = bfpool.tile([P128, BLK, BG], bf16, tag="fbb")
                nc.sync.dma_start(fbb[:], feats_d[:, t0b : t0b + BLK, :])
                efb = bepool.tile([P128, BLK, BG], bf16, tag="efb")
                nc.scalar.activation(efb[:], fbb[:], AF.Exp, bias=biasz[:])

                for k in range(BLK):
                    s = blk * BLK + k
                    # forward step t = s+1: a <- ef_t * (E @ a)
                    pf = fpsum.tile([P128, BG], f32, tag="pf")
                    nc.tensor.matmul(pf[:], wf[:], a_t[:], start=True, stop=True)
                    a_new = apool.tile([P128, BG], bf16, tag="a")
                    nc.vector.tensor_mul(a_new[:], pf[:], eff[:, k, :])
                    a_t = a_new

                    # backward step t = 512-s: v_{t-1} = E^T u_t + delta_{t-1} w
                    # (injection + carrier maintenance ride inside wb)
                    vp = bpsum.tile([P128, BG], f32, tag="vp")
                    nc.tensor.matmul(vp[:], wb[:], u_t[:], start=True, stop=True)
                    if s < M - 1:
                        u_new = upool.tile([P128, BG], bf16, tag="u")
                        nc.vector.tensor_mul(u_new[:], vp[:], efb[:, BLK - 1 - k, :])
                        u_t = u_new
                    else:
                        v256 = vp  # v_256 stays in PSUM for the epilogue

            # Q = sum_{tags>=2}(v256 * a256) per column, r = ln Q + CZ*len
            qprod = accpool.tile([P128, BG], bf16, tag="qprod")
            nc.vector.tensor_mul(qprod[:], v256[:], a_t[:])
            rs = rpool.tile([G, BG], f32, tag="rs")
            nc.tensor.matmul(rs[:], onesg[:], qprod[:], start=True, stop=True)
            logq = accpool.tile([G, BG], f32, tag="logq")
            nc.scalar.activation(logq[:], rs[:], AF.Ln, bias=bias0g[:])
            rout = accpool.tile([G, BG], f32, tag="rout")
            nc.vector.tensor_add(rout[:], logq[:], tw[:])
            nc.sync.dma_start(out_d[:], rout[:])

    nc.compile()
    return nc


def _prep_inputs(feats, mask, transition):
    import ml_dtypes

    bf = ml_dtypes.bfloat16
    feats = np.asarray(feats, dtype=np.float32)
    mask = np.asarray(mask, dtype=np.float32)
    transition = np.asarray(transition, dtype=np.float32)

    lens = mask.sum(axis=0)  # (B,)
    m_pad = np.concatenate([mask, np.zeros((1, B), np.float32)], axis=0)
    # d[t] = m[t-1] - m[t] = 1 iff len == t, for t = 1..512
    d = np.zeros((S + 1, B), np.float32)
    d[1:] = m_pad[:S] - m_pad[1:]

    # forward block-diag weights: wf[g*T+prev, g*T+next] = trans[next, prev]
    wf_log = np.full((G * T, G * T), NEG, np.float32)
    # backward block-diag weights with repurposed carrier rows/cols
    wb_blk = transition.copy()
    wb_blk[0, :] = transition[1, :]   # injection row (delta-carrier contraction)
    wb_blk[1, :] = NEG                # ones row: clear E entries
    wb_blk[:, 0] = NEG                # delta-maintenance col
    wb_blk[:, 1] = NEG                # ones-maintenance col
    wb_blk[1, 1] = 0.0                # ones self-loop
    wb_blk[1, 0] = 0.0                # vp[0-col] = u[ones] = 1
    wb_log = np.full((G * T, G * T), NEG, np.float32)
    for g in range(G):
        wf_log[g * T : (g + 1) * T, g * T : (g + 1) * T] = transition.T
        wb_log[g * T : (g + 1) * T, g * T : (g + 1) * T] = wb_blk

    # feats with carrier rows: tag0 = delta encode, tag1 = exact CZ
    f2 = feats.copy()
    f2[:, :, 1] = CZ
    f2[:, :, 0] = np.where(d[:S] == 1.0, np.float32(CZ), np.float32(NEG))

    w = np.exp(transition[1, :].astype(np.float64))
    ef_last = np.exp(feats[S - 1].astype(np.float64) - CZ)   # (B, T)
    u0_full = (ef_last * d[S][:, None] * w[None, :]).astype(np.float32)
    u0_full[:, 0] = d[S - 1]   # delta-carrier holds d[511]
    u0_full[:, 1] = 1.0        # ones-carrier

    in_maps = []
    for c in range(NCORES):
        sl = slice(c * BL, (c + 1) * BL)
        fc = f2[:, sl, :]  # (S, BL, T)
        fp = np.ascontiguousarray(
            fc.reshape(S, G, BG, T).transpose(1, 3, 0, 2).reshape(G * T, S, BG)
        ).astype(bf)
        u0c = np.ascontiguousarray(
            u0_full[sl].reshape(G, BG, T).transpose(0, 2, 1).reshape(G * T, BG)
        ).astype(bf)
        in_maps.append(
            {
                "feats_t": fp,
                "wf_log": wf_log,
                "wb_log": wb_log,
                "u0": u0c,
                "tw": np.ascontiguousarray(
                    (CZ * lens[sl]).astype(np.float32).reshape(G, BG)
                ),
            }
        )
    return in_maps


def kernel(feats, mask, transition, trace=False):
    global LAST_EXEC_NS
    if "nc" not in _cache:
        _cache["nc"] = _build()
    nc = _cache["nc"]

    in_maps = _prep_inputs(feats, mask, transition)

    from concourse.bass_utils import run_bass_kernel_spmd

    res = run_bass_kernel_spmd(nc, in_maps, core_ids=list(range(NCORES)), trace=trace)
    LAST_EXEC_NS = res.exec_time_ns
    out = np.concatenate([r["out"].reshape(BL) for r in res.results], axis=0)
    return out.astype(np.float32)


# revision 8
# speedup vs baseline: 2.2339x; 1.0039x over previous
"""CRF loss kernel, K=4 rank-1 chunked variant (depth 128 instead of 256).

The sequence [0,512] splits into four 128-step chunks.  Each chunk's transfer
operator M_i = prod_t D_t E is numerically rank-1 (E is a small perturbation of
the all-ones matrix; non-dominant directions contract ~64x per step, so over 128
steps the residual is ~1e-80): M_i ~ u_i q^T / n_i with u_i = M_i p,
v_i = M_i^T q, n_i = q^T u_i, for ANY positive seeds p, q.

Chains (each 64 batch cols per group, 2 groups packed on 128 partitions):
  fwd pack [a | u1 | u2]  (192 cols):  x <- ef * (E x)
    a:  one-hot START init, chunk0 ef idx r        -> a_128
    u1: ones init,          chunk1 ef idx 128+r    -> u1 = M1 p
    u2: ones init,          chunk2 ef idx 256+r    -> u2 = M2 p
  bwd pack [v1 | v2 | h2 | h3] (256 cols):  x <- ef * (E^T x + inj)
    v1: seed ef[255] (chunk-end D folded in), ef idx 254-r, final mm -> v1 = M1^T q
    v2: seed ef[383],                         ef idx 382-r, final mm -> v2 = M2^T q
    h2: delta-injected over [256,384):  init 0 + carrier d[383], final inj d[256]
    h3: delta-injected over [384,512]:  init ef511*d512*w + carrier d[511], final inj d[384]
  delta injection rides in wb's repurposed dead rows (tags 0/1) with delta data
  delivered through feats rows, exactly as the 2-chain kernel.

Stitch:  s1 = v1.a_128, n_i = sum(u_i), P21 = v2.u1, H21 = h2.u1, H32 = h3.u2
  (all dots over live tags >= 2);  Q = (s1/n1) * (H21 + H32*P21/n2);
  r = ln Q + CZ*len.
"""

import sys

import numpy as np

sys.path.insert(0, "/opt/trn_rl_repo")

S, B, T = 512, 1024, 64
NCORES = 8
BL = B // NCORES
G = 2
BG = BL // G       # 64
CZ = 4.6875        # bf16-exact
R = 128            # rounds (chunk length)
BLK = 16
NEG = -10000.0
WF_C = 3 * BG      # fwd pack cols = 192
WB_C = 4 * BG      # bwd pack cols = 256

_cache: dict = {}
LAST_EXEC_NS = None


def _build():
    import concourse.bacc as bacc
    import concourse.bass as bass
    import concourse.mybir as mybir
    import concourse.tile as tile

    f32 = mybir.dt.float32
    bf16 = mybir.dt.bfloat16
    AF = mybir.ActivationFunctionType
    ALU = mybir.AluOpType

    nc = bacc.Bacc("TRN2", target_bir_lowering=False, debug=False, enable_asserts=False)

    P128 = G * T

    fpk_d = nc.dram_tensor("fpk", (P128, R, WF_C), bf16, kind="ExternalInput")
    bpk_d = nc.dram_tensor("bpk", (P128, R, WB_C), bf16, kind="ExternalInput")
    binit_d = nc.dram_tensor("binit", (P128, WB_C), bf16, kind="ExternalInput")
    wf_d = nc.dram_tensor("wf_log", (P128, P128), f32, kind="ExternalInput")
    wb_d = nc.dram_tensor("wb_log", (P128, P128), f32, kind="ExternalInput")
    tw_d = nc.dram_tensor("tw", (G, BG), f32, kind="ExternalInput")
    out_d = nc.dram_tensor("out", (G, BG), f32, kind="ExternalOutput")

    with tile.TileContext(nc) as tc:
        with (
            tc.tile_pool(name="const", bufs=1) as cpool,
            tc.tile_pool(name="ffeat", bufs=3) as ffpool,
            tc.tile_pool(name="fef", bufs=3) as fepool,
            tc.tile_pool(name="bfeat", bufs=3) as bfpool,
            tc.tile_pool(name="bef", bufs=3) as bepool,
            tc.tile_pool(name="xf", bufs=3) as xfpool,
            tc.tile_pool(name="xb", bufs=3) as xbpool,
            tc.tile_pool(name="acc", bufs=1) as accpool,
            tc.tile_pool(name="fps", bufs=3, space=bass.MemorySpace.PSUM) as fpsum,
            tc.tile_pool(name="bps", bufs=3, space=bass.MemorySpace.PSUM) as bpsum,
            tc.tile_pool(name="rp", bufs=1, space=bass.MemorySpace.PSUM) as rpool,
        ):
            bias0 = cpool.tile([P128, 1], f32, tag="bias0")
            nc.vector.memset(bias0[:], 0.0)
            biasz = cpool.tile([P128, 1], f32, tag="biasz")
            nc.vector.memset(biasz[:], -CZ)
            bias0g = cpool.tile([G, 1], f32, tag="bias0g")
            nc.vector.memset(bias0g[:], 0.0)

            wf_log = cpool.tile([P128, P128], f32, tag="wf_log")
            nc.sync.dma_start(wf_log[:], wf_d[:])
            wf = cpool.tile([P128, P128], bf16, tag="wf")
            nc.scalar.activation(wf[:], wf_log[:], AF.Exp, bias=bias0[:])

            wb_log = cpool.tile([P128, P128], f32, tag="wb_log")
            nc.sync.dma_start(wb_log[:], wb_d[:])
            wb = cpool.tile([P128, P128], bf16, tag="wb")
            nc.scalar.activation(wb[:], wb_log[:], AF.Exp, bias=bias0[:])

            tw = cpool.tile([G, BG], f32, tag="tw")
            nc.sync.dma_start(tw[:], tw_d[:])

            # live-tag group-sum weights (exclude carrier rows 0,1 per group)
            onesg = cpool.tile([P128, G], bf16, tag="onesg")
            nc.vector.memset(onesg[:], 0.0)
            nc.vector.memset(onesg[0:T, 0:1], 1.0)
            nc.vector.memset(onesg[T : 2 * T, 1:2], 1.0)
            nc.vector.memset(onesg[0:2, 0:1], 0.0)
            nc.vector.memset(onesg[T : T + 2, 1:2], 0.0)

            # fwd state init: a-cols one-hot START, u-cols all ones
            x_f = xfpool.tile([P128, WF_C], bf16, tag="xf")
            nc.vector.memset(x_f[:], 0.0)
            nc.vector.memset(x_f[:, BG:WF_C], 1.0)
            nc.vector.memset(x_f[0:1, 0:BG], 1.0)
            nc.vector.memset(x_f[T : T + 1, 0:BG], 1.0)

            x_b = xbpool.tile([P128, WB_C], bf16, tag="xb")
            nc.sync.dma_start(x_b[:], binit_d[:])

            vbfin = None
            for blk in range(R // BLK):
                r0 = blk * BLK
                fbf = ffpool.tile([P128, BLK, WF_C], bf16, tag="fbf")
                nc.sync.dma_start(fbf[:], fpk_d[:, r0 : r0 + BLK, :])
                eff = fepool.tile([P128, BLK, WF_C], bf16, tag="eff")
                nc.scalar.activation(eff[:], fbf[:], AF.Exp, bias=biasz[:])

                fbb = bfpool.tile([P128, BLK, WB_C], bf16, tag="fbb")
                nc.sync.dma_start(fbb[:], bpk_d[:, r0 : r0 + BLK, :])
                efb = bepool.tile([P128, BLK, WB_C], bf16, tag="efb")
                nc.scalar.activation(efb[:], fbb[:], AF.Exp, bias=biasz[:])

                for k in range(BLK):
                    r = r0 + k
                    pmf = fpsum.tile([P128, WF_C], f32, tag="pmf")
                    nc.tensor.matmul(pmf[:], wf[:], x_f[:], start=True, stop=True)
                    xf_new = xfpool.tile([P128, WF_C], bf16, tag="xf")
                    nc.vector.tensor_mul(xf_new[:], pmf[:], eff[:, k, :])
                    x_f = xf_new

                    pmb = bpsum.tile([P128, WB_C], f32, tag="pmb")
                    nc.tensor.matmul(pmb[:], wb[:], x_b[:], start=True, stop=True)
                    if r < R - 1:
                        xb_new = xbpool.tile([P128, WB_C], bf16, tag="xb")
                        nc.vector.tensor_mul(xb_new[:], pmb[:], efb[:, k, :])
                        x_b = xb_new
                    else:
                        vbfin = pmb  # [v1 | v2 | h2 | h3] final (PSUM)

            # ---- stitch ----
            macc = accpool.tile([P128, 6, BG], bf16, tag="macc")
            nc.vector.tensor_mul(macc[:, 0, :], vbfin[:, 0:BG], x_f[:, 0:BG])            # s1 = v1*a
            nc.vector.tensor_mul(macc[:, 1, :], vbfin[:, BG : 2 * BG], x_f[:, BG : 2 * BG])   # P21 = v2*u1
            nc.vector.tensor_mul(macc[:, 2, :], vbfin[:, 2 * BG : 3 * BG], x_f[:, BG : 2 * BG])  # H21 = h2*u1
            nc.vector.tensor_mul(macc[:, 3, :], vbfin[:, 3 * BG : 4 * BG], x_f[:, 2 * BG : 3 * BG])  # H32 = h3*u2
            nc.vector.tensor_copy(macc[:, 4, :], x_f[:, BG : 2 * BG])                    # u1
            nc.vector.tensor_copy(macc[:, 5, :], x_f[:, 2 * BG : 3 * BG])                # u2

            rsum = rpool.tile([G, 6, BG], f32, tag="rsum")
            nc.tensor.matmul(rsum[:], onesg[:], macc[:], start=True, stop=True)
            rsb = accpool.tile([G, 6, BG], f32, tag="rsb")
            nc.vector.tensor_copy(rsb[:], rsum[:])

            # log-domain stitch (no divides): Q = (s1/n1)*(H21 + exp(lnH32+lnP21-lnn2))
            lnall = accpool.tile([G, 6, BG], f32, tag="lnall")
            nc.scalar.activation(lnall[:], rsb[:], AF.Ln, bias=bias0g[:])
            t1 = accpool.tile([G, BG], f32, tag="t1")
            nc.vector.tensor_add(t1[:], lnall[:, 3, :], lnall[:, 1, :])    # lnH32+lnP21
            t2 = accpool.tile([G, BG], f32, tag="t2")
            nc.vector.tensor_sub(t2[:], t1[:], lnall[:, 5, :])             # -lnn2
            t3 = accpool.tile([G, BG], f32, tag="t3")
            nc.scalar.activation(t3[:], t2[:], AF.Exp, bias=bias0g[:])     # H32*P21/n2
            tC = accpool.tile([G, BG], f32, tag="tC")
            nc.vector.tensor_add(tC[:], t3[:], rsb[:, 2, :])               # +H21
            lnC = accpool.tile([G, BG], f32, tag="lnC")
            nc.scalar.activation(lnC[:], tC[:], AF.Ln, bias=bias0g[:])
            t4 = accpool.tile([G, BG], f32, tag="t4")
            nc.vector.tensor_add(t4[:], lnC[:], lnall[:, 0, :])            # +lns1
            t5 = accpool.tile([G, BG], f32, tag="t5")
            nc.vector.tensor_sub(t5[:], t4[:], lnall[:, 4, :])             # -lnn1
            rout = accpool.tile([G, BG], f32, tag="rout")
            nc.vector.tensor_add(rout[:], t5[:], tw[:])
            nc.sync.dma_start(out_d[:], rout[:])

    nc.compile()
    return nc


def _pack_core(x, sl):
    # x: (B_sl..., T) for one time index restricted to core slice -> [128, 64]
    return np.ascontiguousarray(
        x.reshape(G, BG, T).transpose(0, 2, 1).reshape(G * T, BG)
    )


def _prep_inputs(feats, mask, transition):
    import ml_dtypes

    bf = ml_dtypes.bfloat16
    feats = np.asarray(feats, dtype=np.float32)
    mask = np.asarray(mask, dtype=np.float32)
    transition = np.asarray(transition, dtype=np.float32)

    lens = mask.sum(axis=0)
    m_pad = np.concatenate([mask, np.zeros((1, B), np.float32)], axis=0)
    d = np.zeros((S + 1, B), np.float32)
    d[1:] = m_pad[:S] - m_pad[1:]

    wf_log = np.full((G * T, G * T), NEG, np.float32)
    wb_blk = transition.copy()
    wb_blk[0, :] = transition[1, :]
    wb_blk[1, :] = NEG
    wb_blk[:, 0] = NEG
    wb_blk[:, 1] = NEG
    wb_blk[1, 1] = 0.0
    wb_blk[1, 0] = 0.0
    wb_log = np.full((G * T, G * T), NEG, np.float32)
    for g in range(G):
        wf_log[g * T : (g + 1) * T, g * T : (g + 1) * T] = transition.T
        wb_log[g * T : (g + 1) * T, g * T : (g + 1) * T] = wb_blk

    # two feats variants: v (row0 = no-inject), h (row0 = delta encode); row1 = CZ
    f2v = feats.copy()
    f2v[:, :, 1] = CZ
    f2v[:, :, 0] = NEG
    f2h = feats.copy()
    f2h[:, :, 1] = CZ
    f2h[:, :, 0] = np.where(d[:S] == 1.0, np.float32(CZ), np.float32(NEG))

    w64 = np.exp(transition[1, :].astype(np.float64))
    ef255 = np.exp(feats[255].astype(np.float64) - CZ)   # (B, T)
    ef383 = np.exp(feats[383].astype(np.float64) - CZ)
    ef511 = np.exp(feats[511].astype(np.float64) - CZ)

    rr = np.arange(R)
    fwd_idx = [rr, 128 + rr, 256 + rr]                  # a, u1, u2
    bwd_idx_v = [254 - rr, 382 - rr]   # v1, v2 (r=127 slot unused by the TT)
    bwd_idx_h = [382 - rr, 510 - rr]   # h2, h3

    in_maps = []
    for c in range(NCORES):
        sl = slice(c * BL, (c + 1) * BL)
        # packed per-time views: pv/ph [128, S, 64]
        pv = np.ascontiguousarray(
            f2v[:, sl, :].reshape(S, G, BG, T).transpose(1, 3, 0, 2).reshape(G * T, S, BG)
        )
        ph = np.ascontiguousarray(
            f2h[:, sl, :].reshape(S, G, BG, T).transpose(1, 3, 0, 2).reshape(G * T, S, BG)
        )
        fpk = np.empty((G * T, R, WF_C), np.float32)
        for j, idx in enumerate(fwd_idx):
            fpk[:, :, j * BG : (j + 1) * BG] = pv[:, idx, :]
        bpk = np.empty((G * T, R, WB_C), np.float32)
        for j, idx in enumerate(bwd_idx_v):
            bpk[:, :, j * BG : (j + 1) * BG] = pv[:, idx, :]
        for j, idx in enumerate(bwd_idx_h):
            bpk[:, :, (2 + j) * BG : (3 + j) * BG] = ph[:, idx, :]

        # binit: [v1 | v2 | h2 | h3]
        binit = np.zeros((G * T, WB_C), np.float32)
        e255 = ef255[sl].copy(); e255[:, 0] = 0.0; e255[:, 1] = 1.0
        binit[:, 0:BG] = _pack_core(e255.astype(np.float32), sl)
        e383 = ef383[sl].copy(); e383[:, 0] = 0.0; e383[:, 1] = 1.0
        binit[:, BG : 2 * BG] = _pack_core(e383.astype(np.float32), sl)
        h2i = np.zeros((BL, T), np.float32)
        h2i[:, 0] = d[383, sl]; h2i[:, 1] = 1.0
        binit[:, 2 * BG : 3 * BG] = _pack_core(h2i, sl)
        h3i = (ef511[sl] * d[S, sl][:, None] * w64[None, :]).astype(np.float32)
        h3i[:, 0] = d[S - 1, sl]; h3i[:, 1] = 1.0
        binit[:, 3 * BG : 4 * BG] = _pack_core(h3i, sl)

        in_maps.append(
            {
                "fpk": fpk.astype(bf),
                "bpk": bpk.astype(bf),
                "binit": binit.astype(bf),
                "wf_log": wf_log,
                "wb_log": wb_log,
                "tw": np.ascontiguousarray(
                    (CZ * lens[sl]).astype(np.float32).reshape(G, BG)
                ),
            }
        )
    return in_maps


def kernel(feats, mask, transition, trace=False):
    global LAST_EXEC_NS
    if "nc" not in _cache:
        _cache["nc"] = _build()
    nc = _cache["nc"]

    in_maps = _prep_inputs(feats, mask, transition)

    from concourse.bass_utils import run_bass_kernel_spmd

    res = run_bass_kernel_spmd(nc, in_maps, core_ids=list(range(NCORES)), trace=trace)
    LAST_EXEC_NS = res.exec_time_ns
    out = np.concatenate([r["out"].reshape(BL) for r in res.results], axis=0)
    return out.astype(np.float32)
